# revision 4
# baseline (speedup 1.0000x reference)
"""3-layer GAT + linear head on 8 TRN2 NeuronCores (Bass/Tile) — v2.

Key design vs v1:
  - bf16 node tables, gathers, and matmuls everywhere (PSUM accumulation
    stays f32); tolerance is 2e-2 so bf16 is safe.
  - Layer 1 needs no collective: every core computes the FULL H1 = X@W1
    table (83us of PE) into its local HBM and gathers locally, instead of
    AllGathering a 115MB f32 table.
  - Layers 2/3 AllGather small bf16 tables ([50000,128] = 12.8MB).
  - Ragged per-tile chunk counts (max over cores, SPMD) with -1-padded
    gather indices: pad rows are skipped by the DMA (no wasted bytes).
  - The transposed one-hot matrices (s01t, for the a_dst edge broadcast)
    are precomputed on host and streamed from HBM (one DMA per tile),
    removing the per-chunk PE transpose + PSUM copy of v1.
  - One-hot s01 built on device with a single DVE is_equal per chunk; for
    1-head layers the edge weight w is fused into it (is_equal * w), so
    no separate weighting pass is needed.
  - Transposed activations (hT) stay SBUF-resident between layers.

Self-contained; hardcodes N=50000, E=800000, D_IN=128, HID=64, HEADS=8,
D_OUT=10, 8 cores.
"""
import os
import numpy as np
import ml_dtypes

import concourse.bass as bass
import concourse.mybir as mybir
import concourse.tile as tile
from concourse import bacc
from concourse.bass_utils import run_bass_kernel_spmd
from concourse.masks import make_identity

N = 50000
E = 800000
NCORES = 8
VP = N // NCORES          # 6250
P = 128
NT = (VP + P - 1) // P    # 49 dst tiles per core
NTP = NT * P              # 6272
NG = (N + P - 1) // P     # 391 global tiles
NGP = NG * P              # 50048
HALF = N // 2             # 25000 (A/B split value)
BASE_B = N - 32768        # 17232 (offset of the B half table slice)
D_IN = 128
HID = 64
HEADS = 8
D_OUT = 10
R1 = 640                  # layer-1 row: h(512) | a_s(8) | pad -> 1280B
R2 = 128                  # layer-2/3 row: h(64) | a_s(1) | pad -> 256B

f32 = mybir.dt.float32
bf16 = mybir.dt.bfloat16
i16 = mybir.dt.int16
AT = mybir.AluOpType
AF = mybir.ActivationFunctionType
BF = ml_dtypes.bfloat16

PHASES = ["m1", "e1", "m2", "ag2", "e2", "m3", "ag3", "e3", "fin"]


def _prep_edges(edge_index):
    """Returns (meta, idx_res, dst_res, s01t) where
    meta[t] = (cA, cB, col_base) uniform across cores,
    idx_res[k]: [128, TOTC*8] i16, dst_res[k]: [128, TOTC] f32,
    s01t[k]: [128, TOTC*128] bf16."""
    src = np.concatenate([np.asarray(edge_index[0]),
                          np.arange(N)]).astype(np.int64)
    dst = np.concatenate([np.asarray(edge_index[1]),
                          np.arange(N)]).astype(np.int64)

    per_core = []
    nA = np.zeros((NCORES, NT), np.int64)
    nB = np.zeros((NCORES, NT), np.int64)
    for k in range(NCORES):
        m = (dst >= k * VP) & (dst < (k + 1) * VP)
        s_k = src[m]
        dloc = dst[m] - k * VP
        t_k = dloc // P
        w_k = dloc % P
        tiles = []
        for t in range(NT):
            sel = t_k == t
            ss, ww = s_k[sel], w_k[sel]
            ga = ss < HALF
            A = (ss[ga], ww[ga])
            B = (ss[~ga] - BASE_B, ww[~ga])
            nA[k, t] = len(A[0])
            nB[k, t] = len(B[0])
            tiles.append((A, B))
        per_core.append(tiles)

    cA = np.maximum((nA.max(0) + P - 1) // P, 0).astype(int)
    cB = np.maximum((nB.max(0) + P - 1) // P, 0).astype(int)
    meta = []
    base = 0
    for t in range(NT):
        meta.append((int(cA[t]), int(cB[t]), base))
        base += int(cA[t]) + int(cB[t])
    totc = base

    idx_res, dst_res, s01t_res = [], [], []
    for k in range(NCORES):
        idx = np.zeros((16, totc * 8), np.int16)
        dstv = np.full((128, totc), -1.0, np.float32)
        s01t = np.zeros((128, totc * 128), BF)
        for t in range(NT):
            ca, cb, cb0 = meta[t]
            for g, cg in ((0, ca), (1, cb)):
                li, ww = per_core[k][t][g]
                o = cb0 if g == 0 else cb0 + ca
                n = len(li)
                if n == 0:
                    continue
                i = np.arange(n)
                idx[i % 16, o * 8 + i // 16] = li.astype(np.int16)
                dstv[i % 128, o + i // 128] = ww
                s01t[ww, (o + i // 128) * 128 + (i % 128)] = 1.0
        idx_res.append(np.tile(idx, (8, 1)))
        dst_res.append(dstv)
        s01t_res.append(s01t)
    return meta, totc, idx_res, dst_res, s01t_res


def _edge_phase(nc, tc, layer, meta, hfull, ad_sb, iota, brep, ones_col,
                hT_out, R, C, H):
    """GAT edge aggregation for one layer over the core's 49 dst tiles."""
    CAS = 512 if H == 8 else 64       # a_s column in the gathered row
    ghmax = max(max(m[0], m[1]) for m in meta)
    ctmax = max(m[0] + m[1] for m in meta)
    s01t_d = nc.t.s01t_d
    idx_sb, dst_sb = nc._idx_sb, nc._dst_sb

    with tc.tile_pool(name=f"e{layer}g", bufs=3) as gp, \
         tc.tile_pool(name=f"e{layer}w", bufs=3) as wp, \
         tc.tile_pool(name=f"e{layer}s", bufs=3) as sp, \
         tc.tile_pool(name=f"e{layer}t", bufs=3) as stp, \
         tc.tile_pool(name=f"e{layer}p1", bufs=2, space="PSUM") as pp, \
         tc.tile_pool(name=f"e{layer}p2", bufs=2, space="PSUM") as p2, \
         tc.tile_pool(name=f"e{layer}p3", bufs=2, space="PSUM") as p3:
        # prime gather-target tiles so pad rows hold finite values
        for _ in range(3):
            gt = gp.tile([P, ghmax, R], bf16, tag="G")
            nc.gpsimd.memset(gt[:], 0)
        for t in range(NT):
            ca, cb, cb0 = meta[t]
            cht = ca + cb
            s01t_sb = stp.tile([P, ctmax * P], bf16, tag="s01t")
            nc.sync.dma_start(out=s01t_sb[:, 0:cht * P],
                              in_=s01t_d[:, cb0 * P:(cb0 + cht) * P])
            outu = p2.tile([P, C], f32, space="PSUM", tag="outu")
            ssum = p2.tile([P, H], f32, space="PSUM", tag="ssum")
            est = wp.tile([P, ctmax * H], bf16 if H == 8 else f32,
                          tag="est")
            pos = 0
            for g, cg in ((0, ca), (1, cb)):
                if cg == 0:
                    continue
                G = gp.tile([P, ghmax, R], bf16, tag="G")
                nc.gpsimd.dma_gather(
                    G[:, 0:cg, :], hfull[g], idx_sb[:, (cb0 + pos) * 8:
                                                    (cb0 + pos + cg) * 8],
                    cg * P, cg * P, R, single_packet=False, queue_num=g)
                adg_all = pp.tile([P, ctmax * H], f32, space="PSUM",
                                  tag="adg")
                for c in range(cg):
                    pc = pos + c
                    nc.tensor.matmul(
                        adg_all[:, c * H:(c + 1) * H],
                        lhsT=s01t_sb[:, pc * P:(pc + 1) * P],
                        rhs=ad_sb[:, t * H:(t + 1) * H],
                        start=True, stop=True)
                ev = est[:, pos * H:(pos + cg) * H].rearrange(
                    "p (c h) -> p c h", c=cg)
                av = adg_all[:, 0:cg * H].rearrange("p (c h) -> p c h", c=cg)
                nc.vector.tensor_tensor(
                    out=ev, in0=G[:, 0:cg, CAS:CAS + H], in1=av, op=AT.add)
                ef = est[:, pos * H:(pos + cg) * H]
                nc.vector.scalar_tensor_tensor(
                    out=ef, in0=ef, scalar=0.2, in1=ef,
                    op0=AT.mult, op1=AT.max)
                nc.scalar.activation(ef, ef, AF.Exp)
                for c in range(cg):
                    pc = pos + c
                    first = pc == 0
                    last = pc == cht - 1
                    col = cb0 + pc
                    s01 = sp.tile([P, P], bf16, tag="s01")
                    if H == 1:
                        # one-hot scaled by the edge weight in one DVE op
                        nc.vector.tensor_scalar(
                            out=s01[:], in0=iota[:],
                            scalar1=dst_sb[:, col:col + 1],
                            scalar2=est[:, pc:pc + 1],
                            op0=AT.is_equal, op1=AT.mult)
                        rhs_s = ones_col
                    else:
                        nc.vector.tensor_scalar(
                            out=s01[:], in0=iota[:],
                            scalar1=dst_sb[:, col:col + 1],
                            scalar2=None, op0=AT.is_equal)
                        gv = G[:, c, 0:C].rearrange("p (h c) -> p h c", h=H)
                        nc.vector.tensor_tensor(
                            out=gv, in0=gv,
                            in1=est[:, pc * H:(pc + 1) * H].to_broadcast(
                                [P, H, C // H]),
                            op=AT.mult)
                        rhs_s = est[:, pc * H:(pc + 1) * H]
                    nc.tensor.matmul(outu[:], lhsT=s01[:],
                                     rhs=G[:, c, 0:C],
                                     start=first, stop=last,
                                     skip_group_check=True)
                    nc.tensor.matmul(ssum[:], lhsT=s01[:], rhs=rhs_s,
                                     start=first, stop=last,
                                     skip_group_check=True)
                pos += cg
            # epilogue: normalize, bias, ELU, transpose into hT_out
            rec = wp.tile([P, H], f32, tag="rec")
            nc.vector.reciprocal(rec[:], ssum[:])
            ho = wp.tile([P, C], bf16, tag="ho")
            if H == 8:
                hv = ho[:].rearrange("p (h c) -> p h c", h=H)
                ov = outu[:].rearrange("p (h c) -> p h c", h=H)
                nc.vector.tensor_tensor(
                    out=hv, in0=ov,
                    in1=rec[:].to_broadcast([P, H, C // H]), op=AT.mult)
            else:
                nc.vector.tensor_scalar(
                    out=ho[:], in0=outu[:], scalar1=rec[:, 0:1],
                    scalar2=None, op0=AT.mult)
            nc.vector.tensor_tensor(out=ho[:], in0=ho[:], in1=brep[:],
                                    op=AT.add)
            # ELU negative branch in f32: bf16 exp(x)-1 cancels near 0
            el = wp.tile([P, C], f32, tag="el")
            nc.vector.tensor_scalar(out=el[:], in0=ho[:], scalar1=0.0,
                                    scalar2=None, op0=AT.min)
            nc.scalar.activation(el[:], el[:], AF.Exp)
            nc.vector.tensor_scalar(out=el[:], in0=el[:], scalar1=-1.0,
                                    scalar2=None, op0=AT.add)
            nc.vector.scalar_tensor_tensor(
                out=ho[:], in0=ho[:], scalar=0.0, in1=el[:],
                op0=AT.max, op1=AT.add)
            if C == 512:
                for cbk in range(4):
                    tp = p3.tile([P, P], bf16, space="PSUM", tag="tp")
                    nc.tensor.transpose(out=tp[:],
                                        in_=ho[:, cbk * P:(cbk + 1) * P],
                                        identity=nc._ident_bf[:])
                    nc.scalar.activation(
                        hT_out[:, cbk * NTP + t * P:cbk * NTP + (t + 1) * P],
                        tp[:], AF.Copy)
            else:
                tp = p3.tile([P, P], bf16, space="PSUM", tag="tp")
                nc.tensor.transpose(out=tp[:HID, :], in_=ho[:],
                                    identity=nc._ident_bf[:])
                nc.scalar.activation(hT_out[:, t * P:(t + 1) * P],
                                     tp[:HID, :], AF.Copy)


def _build_program(meta, totc):
    stop = os.environ.get("GATV2_STOP", "fin")
    lvl = PHASES.index(stop) + 1
    nc = bacc.Bacc("TRN2", target_bir_lowering=False, debug=False,
                   enable_asserts=False, num_devices=NCORES,
                   num_swdge_queues=2)

    class T:
        pass

    nc.t = T()
    xT_in = nc.dram_tensor("xT", [P, NGP], bf16, kind="ExternalInput")
    xTo_in = nc.dram_tensor("xTo", [P, NTP], bf16, kind="ExternalInput")
    idx_in = nc.dram_tensor("idx", [P, totc * 8], i16, kind="ExternalInput")
    dst_in = nc.dram_tensor("dst", [P, totc], f32, kind="ExternalInput")
    nc.t.s01t_d = nc.dram_tensor("s01t", [P, totc * P], bf16,
                                 kind="ExternalInput")
    W1T_in = nc.dram_tensor("W1T", [D_IN, 512], bf16, kind="ExternalInput")
    M1s_in = nc.dram_tensor("M1s", [D_IN, 8], bf16, kind="ExternalInput")
    M1d_in = nc.dram_tensor("M1d", [D_IN, 8], bf16, kind="ExternalInput")
    rhs2_in = nc.dram_tensor("rhs2", [P, 4 * 66], bf16, kind="ExternalInput")
    rhs3_in = nc.dram_tensor("rhs3", [HID, 66], bf16, kind="ExternalInput")
    rhsc_in = nc.dram_tensor("rhsc", [HID, D_OUT], bf16,
                             kind="ExternalInput")
    b1r_in = nc.dram_tensor("b1r", [P, 512], bf16, kind="ExternalInput")
    b2r_in = nc.dram_tensor("b2r", [P, HID], bf16, kind="ExternalInput")
    b3r_in = nc.dram_tensor("b3r", [P, HID], bf16, kind="ExternalInput")
    bcr_in = nc.dram_tensor("bcr", [P, D_OUT], f32, kind="ExternalInput")

    out_d = nc.dram_tensor("out", [NTP, D_OUT], f32, kind="ExternalOutput")
    debug = os.environ.get("GATV2_DEBUG") == "1"
    if debug:
        dbg_hc1 = nc.dram_tensor("dbg_hc1", [256, R1], bf16,
                                 kind="ExternalOutput")
        dbg_ad1 = nc.dram_tensor("dbg_ad1", [P, NT * 8], bf16,
                                 kind="ExternalOutput")
        dbg_hT1 = nc.dram_tensor("dbg_hT1", [P, 4 * NTP], bf16,
                                 kind="ExternalOutput")
        dbg_hT2 = nc.dram_tensor("dbg_hT2", [HID, NTP], bf16,
                                 kind="ExternalOutput")
        dbg_hT3 = nc.dram_tensor("dbg_hT3", [HID, NTP], bf16,
                                 kind="ExternalOutput")

    hcat1 = nc.dram_tensor("hcat1", [N, R1], bf16, kind="Internal")
    h2loc = nc.dram_tensor("h2loc", [VP, R2], bf16, kind="Internal")
    h2full = nc.dram_tensor("h2full", [N, R2], bf16, kind="Internal",
                            addr_space="Shared")
    h3loc = nc.dram_tensor("h3loc", [VP, R2], bf16, kind="Internal")
    h3full = nc.dram_tensor("h3full", [N, R2], bf16, kind="Internal",
                            addr_space="Shared")

    def rows_of(t):
        return P if t < NT - 1 else VP - (NT - 1) * P

    rg = [list(range(NCORES))]

    with tile.TileContext(nc) as tc:
        with tc.tile_pool(name="const", bufs=1) as cs:
            ident_bf = cs.tile([P, P], bf16)
            make_identity(nc, ident_bf[:])
            nc._ident_bf = ident_bf
            iota = cs.tile([P, P], bf16)
            nc.gpsimd.iota(iota[:], pattern=[[1, P]], base=0,
                           channel_multiplier=0,
                           allow_small_or_imprecise_dtypes=True)
            ones_col = cs.tile([P, 1], bf16)
            nc.gpsimd.memset(ones_col[:], 1.0)

            def c_load(name, shape, src, dt=bf16):
                tl = cs.tile(shape, dt, tag=name)
                nc.sync.dma_start(out=tl[:], in_=src)
                return tl

            W1T = c_load("W1T", [D_IN, 512], W1T_in[:])
            M1s = c_load("M1s", [D_IN, 8], M1s_in[:])
            M1d = c_load("M1d", [D_IN, 8], M1d_in[:])
            rhs2 = c_load("rhs2", [P, 4 * 66], rhs2_in[:])
            rhs3 = c_load("rhs3", [HID, 66], rhs3_in[:])
            rhsc = c_load("rhsc", [HID, D_OUT], rhsc_in[:])
            b1r = c_load("b1r", [P, 512], b1r_in[:])
            b2r = c_load("b2r", [P, HID], b2r_in[:])
            b3r = c_load("b3r", [P, HID], b3r_in[:])
            bcr = c_load("bcr", [P, D_OUT], bcr_in[:], dt=f32)
            idx_sb = c_load("idxr", [P, totc * 8], idx_in[:], dt=i16)
            dst_sb = c_load("dstr", [P, totc], dst_in[:], dt=f32)
            nc._idx_sb, nc._dst_sb = idx_sb, dst_sb

            ad1 = cs.tile([P, NT * 8], bf16)
            ad2 = cs.tile([P, NT], bf16)
            ad3 = cs.tile([P, NT], bf16)
            hT1 = cs.tile([P, 4 * NTP], bf16)
            hT2 = cs.tile([HID, NTP], bf16)
            hT3 = cs.tile([HID, NTP], bf16)

            # ---- M1: full H1 table for all N nodes + own a_d ----
            if lvl >= 1:
             with tc.tile_pool(name="m1", bufs=3) as mp, \
                 tc.tile_pool(name="m1p", bufs=2, space="PSUM") as mpp, \
                 tc.tile_pool(name="m1q", bufs=2, space="PSUM") as mpq:
                for _ in range(3):
                    hc = mp.tile([P, R1], bf16, tag="hc")
                    nc.gpsimd.memset(hc[:], 0)
                for j in range(NG):
                    if j % 4 == 0:
                        nsl = min(4, NG - j)
                        xt4 = mp.tile([P, 4 * P], bf16, tag="xt4")
                        nc.sync.dma_start(
                            out=xt4[:, 0:nsl * P],
                            in_=xT_in[:, j * P:(j + nsl) * P])
                    xt = xt4[:, (j % 4) * P:(j % 4 + 1) * P]
                    h_ps = mpp.tile([P, 512], f32, space="PSUM", tag="h")
                    nc.tensor.matmul(h_ps[:], lhsT=xt, rhs=W1T[:],
                                     start=True, stop=True)
                    as_ps = mpq.tile([P, 8], f32, space="PSUM", tag="as")
                    nc.tensor.matmul(as_ps[:], lhsT=xt, rhs=M1s[:],
                                     start=True, stop=True)
                    hc = mp.tile([P, R1], bf16, tag="hc")
                    nc.vector.tensor_copy(hc[:, 0:256], h_ps[:, 0:256])
                    nc.scalar.activation(hc[:, 256:512], h_ps[:, 256:512],
                                         AF.Copy)
                    nc.scalar.activation(hc[:, 512:520], as_ps[:], AF.Copy)
                    r = P if j < NG - 1 else N - (NG - 1) * P
                    nc.sync.dma_start(out=hcat1[j * P:j * P + r, :],
                                      in_=hc[:r, :])
                for t in range(NT):
                    xt = mp.tile([P, P], bf16, tag="xt")
                    nc.sync.dma_start(out=xt[:],
                                      in_=xTo_in[:, t * P:(t + 1) * P])
                    ad_ps = mpq.tile([P, 8], f32, space="PSUM", tag="as")
                    nc.tensor.matmul(ad_ps[:], lhsT=xt[:], rhs=M1d[:],
                                     start=True, stop=True)
                    nc.scalar.activation(ad1[:, t * 8:(t + 1) * 8],
                                         ad_ps[:], AF.Copy)

            if lvl >= 2:
             _edge_phase(nc, tc, 1, meta,
                         (hcat1[0:32768, :], hcat1[BASE_B:N, :]),
                         ad1, iota, b1r, ones_col, hT1, R1, 512, 8)

            # ---- M2 ----
            if lvl >= 3:
             with tc.tile_pool(name="m2", bufs=3) as mp, \
                 tc.tile_pool(name="m2p", bufs=2, space="PSUM") as mpp:
                for _ in range(3):
                    hc = mp.tile([P, R2], bf16, tag="hc2")
                    nc.gpsimd.memset(hc[:], 0)
                for t in range(NT):
                    h_ps = mpp.tile([P, 66], f32, space="PSUM", tag="h")
                    for cbk in range(4):
                        nc.tensor.matmul(
                            h_ps[:],
                            lhsT=hT1[:, cbk * NTP + t * P:
                                     cbk * NTP + (t + 1) * P],
                            rhs=rhs2[:, cbk * 66:(cbk + 1) * 66],
                            start=(cbk == 0), stop=(cbk == 3))
                    hc = mp.tile([P, R2], bf16, tag="hc2")
                    nc.vector.tensor_copy(hc[:, 0:65], h_ps[:, 0:65])
                    nc.scalar.activation(ad2[:, t:t + 1], h_ps[:, 65:66],
                                         AF.Copy)
                    r = rows_of(t)
                    nc.sync.dma_start(out=h2loc[t * P:t * P + r, :],
                                      in_=hc[:r, :])
            if lvl >= 4:
             nc.gpsimd.collective_compute(
                "AllGather", AT.bypass, replica_groups=rg,
                ins=[h2loc[:]], outs=[h2full[:]])

            if lvl >= 5:
             _edge_phase(nc, tc, 2, meta,
                         (h2full[0:32768, :], h2full[BASE_B:N, :]),
                         ad2, iota, b2r, ones_col, hT2, R2, HID, 1)

            # ---- M3 ----
            if lvl >= 6:
             with tc.tile_pool(name="m3", bufs=3) as mp, \
                 tc.tile_pool(name="m3p", bufs=2, space="PSUM") as mpp:
                for _ in range(3):
                    hc = mp.tile([P, R2], bf16, tag="hc3")
                    nc.gpsimd.memset(hc[:], 0)
                for t in range(NT):
                    h_ps = mpp.tile([P, 66], f32, space="PSUM", tag="h")
                    nc.tensor.matmul(h_ps[:],
                                     lhsT=hT2[:, t * P:(t + 1) * P],
                                     rhs=rhs3[:], start=True, stop=True)
                    hc = mp.tile([P, R2], bf16, tag="hc3")
                    nc.vector.tensor_copy(hc[:, 0:65], h_ps[:, 0:65])
                    nc.scalar.activation(ad3[:, t:t + 1], h_ps[:, 65:66],
                                         AF.Copy)
                    r = rows_of(t)
                    nc.sync.dma_start(out=h3loc[t * P:t * P + r, :],
                                      in_=hc[:r, :])
            if lvl >= 7:
             nc.gpsimd.collective_compute(
                "AllGather", AT.bypass, replica_groups=rg,
                ins=[h3loc[:]], outs=[h3full[:]])

            if lvl >= 8:
             _edge_phase(nc, tc, 3, meta,
                         (h3full[0:32768, :], h3full[BASE_B:N, :]),
                         ad3, iota, b3r, ones_col, hT3, R2, HID, 1)

            # ---- final linear ----
            if lvl >= 9:
             with tc.tile_pool(name="fin", bufs=3) as mp, \
                 tc.tile_pool(name="finp", bufs=2, space="PSUM") as mpp:
                for t in range(NT):
                    o_ps = mpp.tile([P, D_OUT], f32, space="PSUM", tag="o")
                    nc.tensor.matmul(o_ps[:],
                                     lhsT=hT3[:, t * P:(t + 1) * P],
                                     rhs=rhsc[:], start=True, stop=True)
                    ob = mp.tile([P, D_OUT], f32, tag="ob")
                    nc.vector.tensor_tensor(out=ob[:], in0=o_ps[:],
                                            in1=bcr[:], op=AT.add)
                    r = rows_of(t)
                    nc.sync.dma_start(out=out_d[t * P:t * P + r, :],
                                      in_=ob[:r, :])
            if debug:
                with tc.tile_pool(name="dbg", bufs=2) as dp:
                    nc.sync.dma_start(out=dbg_ad1[:], in_=ad1[:])
                    nc.sync.dma_start(out=dbg_hT1[:], in_=hT1[:])
                    nc.sync.dma_start(out=dbg_hT2[:], in_=hT2[:])
                    nc.sync.dma_start(out=dbg_hT3[:], in_=hT3[:])
                    for j in range(2):
                        tt = dp.tile([P, R1], bf16, tag="tt")
                        nc.sync.dma_start(out=tt[:],
                                          in_=hcat1[j * P:(j + 1) * P, :])
                        nc.sync.dma_start(out=dbg_hc1[j * P:(j + 1) * P, :],
                                          in_=tt[:])

    nc.compile()
    return nc


def prepare(**inputs):
    x = np.asarray(inputs["x"], np.float32)
    edge_index = np.asarray(inputs["edge_index"])
    W1 = np.asarray(inputs["W1"], np.float32)
    a1_src = np.asarray(inputs["a1_src"], np.float32)
    a1_dst = np.asarray(inputs["a1_dst"], np.float32)
    b1 = np.asarray(inputs["b1"], np.float32)
    W2 = np.asarray(inputs["W2"], np.float32)
    a2_src = np.asarray(inputs["a2_src"], np.float32)
    a2_dst = np.asarray(inputs["a2_dst"], np.float32)
    b2 = np.asarray(inputs["b2"], np.float32)
    W3 = np.asarray(inputs["W3"], np.float32)
    a3_src = np.asarray(inputs["a3_src"], np.float32)
    a3_dst = np.asarray(inputs["a3_dst"], np.float32)
    b3 = np.asarray(inputs["b3"], np.float32)
    Wc = np.asarray(inputs["Wc"], np.float32)
    bc = np.asarray(inputs["bc"], np.float32)

    meta, totc, idx_res, dst_res, s01t_res = _prep_edges(edge_index)

    W1h = W1.reshape(HEADS, HID, D_IN)
    M1s = np.einsum("hci,hc->ih", W1h, a1_src)            # [128, 8]
    M1d = np.einsum("hci,hc->ih", W1h, a1_dst)
    # rhs2 = [W2T | M2s | M2d] -> [512, 66] -> SBUF layout [128, 4*66]
    rhs2 = np.concatenate(
        [W2.T, (W2.T @ a2_src[0])[:, None], (W2.T @ a2_dst[0])[:, None]], 1)
    rhs2_sb = np.zeros((P, 4 * 66), np.float32)
    for cbk in range(4):
        rhs2_sb[:, cbk * 66:(cbk + 1) * 66] = rhs2[cbk * P:(cbk + 1) * P]
    rhs3 = np.concatenate(
        [W3.T, (W3.T @ a3_src[0])[:, None], (W3.T @ a3_dst[0])[:, None]], 1)

    xTg = np.zeros((P, NGP), np.float32)
    xTg[:, :N] = x.T

    common = {
        "W1T": W1.T.astype(BF),
        "M1s": M1s.astype(BF), "M1d": M1d.astype(BF),
        "rhs2": rhs2_sb.astype(BF),
        "rhs3": rhs3.astype(BF),
        "rhsc": Wc.T.astype(BF),
        "b1r": np.tile(b1, (P, 1)).astype(BF),
        "b2r": np.tile(b2, (P, 1)).astype(BF),
        "b3r": np.tile(b3, (P, 1)).astype(BF),
        "bcr": np.tile(bc, (P, 1)).astype(np.float32),
        "xT": xTg.astype(BF),
    }

    in_maps = []
    for k in range(NCORES):
        m = dict(common)
        xo = np.zeros((P, NTP), np.float32)
        xo[:, :VP] = x[k * VP:(k + 1) * VP].T
        m["xTo"] = xo.astype(BF)
        m["idx"] = idx_res[k]
        m["dst"] = dst_res[k]
        m["s01t"] = s01t_res[k]
        in_maps.append(m)

    nc = _build_program(meta, totc)
    return nc, in_maps


def kernel(**inputs):
    nc, in_maps = prepare(**inputs)
    r = run_bass_kernel_spmd(nc, in_maps, core_ids=list(range(NCORES)))
    out = np.concatenate([r.results[k]["out"][:VP] for k in range(NCORES)], 0)
    return out.astype(np.float32)
